# revision 4
# baseline (speedup 1.0000x reference)
"""Trainium2 Bass kernel for an AttentionBlock (GroupNorm + single-head
full N^2 attention + output projection + residual), data-parallel over
batch: 8 samples on 8 NeuronCores, no collectives.

Shapes (hardcoded): x [8, 256, 64, 64]; weights [256, 256]; biases [256].
Per core: one batch sample, x viewed as [C=256, N=4096] channel-major.

Per-core pipeline (fp8e4m3 DoubleRow attention matmuls, bf16 projections):
  1. GroupNorm (8 groups) in C-major layout: per-partition bn_stats,
     cross-partition group reduction via tiny constant matmuls, applied as
     per-partition scale/bias. Rounded tokens t_r are bf16; the fp32
     residual copy (with bp + Wp@bv folded in) runs on the Pool engine,
     overlapped with attention.
  2. Q,K projections (bf16) emit fp8 q',k' pre-scaled by sqrt(A) with
     A = 8*log2(e)/sqrt(C), so the QK^T PSUM value is directly the fp8
     exponent-byte coordinate: byte = psum + BS. The output projection is
     folded into V: v2 = t @ (Wp Wv).T in bf16 -> fp8, with a ones column
     so PV also produces softmax denominators; Wp@bv moves to the final
     residual bias.
  3. Attention per 512-query chunk over 16 key-block *pairs*: one fp8
     DoubleRow matmul per key block (contraction 256 in one instruction,
     0.5 cyc/row); softmax exp is split across engines per pair:
       ACT: out = Exp(ln2/8 * psum + bias) written as fp8e4m3
       DVE: byte = round(max(psum + BS, 0)) as uint8 == the fp8 bit
            pattern of the same alpha*exp(s) (Schraudolph-in-fp8)
     Both paths share the scale alpha = 2^((BS-56)/8), which cancels in
     the softmax normalization. PV accumulates [proj | colsum] in PSUM
     with fp8 DoubleRow over pairs.
  4. Epilogue per chunk: per-128-query reciprocal + ACT copy-scale
     normalize, f32r TensorE transposes back to C-major, DVE residual add
     against the Pool-prepared t_cm, single DMA out per chunk.
"""

import numpy as np

import concourse.bacc as bacc
import concourse.mybir as mybir
import concourse.tile as tile
from concourse import bass_utils

F32 = mybir.dt.float32
F32R = mybir.dt.float32r
BF16 = mybir.dt.bfloat16
FP8 = mybir.dt.float8e4
U8 = mybir.dt.uint8
AF = mybir.ActivationFunctionType
OP = mybir.AluOpType
DR = mybir.MatmulPerfMode.DoubleRow

B = 8
C = 256
H = 64
W = 64
N = H * W  # 4096 tokens
G = 8  # groups
GS = C // G  # 32 channels per group
P = 128
CB = C // P  # 2 channel blocks
EPS = 1e-5
NCHUNK = 512  # query chunk
NJ = N // NCHUNK  # 8
MB = N // P  # 32 key blocks
NP = MB // 2  # 16 key-block pairs
JJ = NCHUNK // P  # 4 query sub-blocks per chunk
VW = 272  # padded v_aug width (DoubleRow pair stride must be %16)
DCOL = 256  # denominator (ones) column index

SQA = float(np.sqrt(8.0 * np.log2(np.e) / 16.0))  # 0.849329
BS = 20.0  # byte bias: byte = psum + BS
LN2_8 = float(np.log(2.0) / 8.0)
EBIAS = float((BS - 56.0) * np.log(2.0) / 8.0)

# exp engine assignment per pair: True -> ACT, False -> DVE
ACT_PAIRS = tuple(p % 2 == 0 for p in range(NP))

_CACHE: dict = {}


def build_nc(att_reps=1, act_pairs=ACT_PAIRS):
    nc = bacc.Bacc(
        "TRN2",
        target_bir_lowering=False,
        debug=False,
        enable_asserts=False,
        num_devices=B,
    )

    x_d = nc.dram_tensor("x", [C, N], F32, kind="ExternalInput")
    gamma_d = nc.dram_tensor("gamma", [C], F32, kind="ExternalInput")
    beta_d = nc.dram_tensor("beta", [C], F32, kind="ExternalInput")
    w_d = {}
    b_d = {}
    for nm in ("q", "k", "v", "p"):
        w_d[nm] = nc.dram_tensor(f"W{nm}", [C, C], F32, kind="ExternalInput")
        b_d[nm] = nc.dram_tensor(f"b{nm}", [C], F32, kind="ExternalInput")
    out_d = nc.dram_tensor("out", [C, N], F32, kind="ExternalOutput")

    ident_d = nc.inline_tensor(np.eye(P, dtype=np.float32), name="ident")
    gsum_np = np.zeros((P, G // CB), np.float32)
    for p in range(P):
        gsum_np[p, p // GS] = 1.0 / GS
    gsum_d = nc.inline_tensor(gsum_np, name="gsum")
    gbc_np = np.zeros((G // CB, P), np.float32)
    for p in range(P):
        gbc_np[p // GS, p] = 1.0
    gbc_d = nc.inline_tensor(gbc_np, name="gbc")

    from contextlib import ExitStack

    with tile.TileContext(nc) as tc:
        with ExitStack() as ctx:
            _build_tile(
                ctx, tc, x_d, gamma_d, beta_d, w_d, b_d, out_d, ident_d,
                gsum_d, gbc_d, att_reps, act_pairs,
            )
    nc.compile()
    return nc


def _build_tile(ctx, tc, x_d, gamma_d, beta_d, w_d, b_d, out_d, ident_d,
                gsum_d, gbc_d, att_reps, act_pairs):
    nc = tc.nc

    persist = ctx.enter_context(tc.tile_pool(name="persist", bufs=1))
    staging = ctx.enter_context(tc.tile_pool(name="staging", bufs=2))
    sexp = ctx.enter_context(tc.tile_pool(name="sexp", bufs=4))
    sout = ctx.enter_context(tc.tile_pool(name="sout", bufs=2))
    stmp = ctx.enter_context(tc.tile_pool(name="stmp", bufs=6))
    ps_sc = ctx.enter_context(tc.tile_pool(name="ps_sc", bufs=2, space="PSUM"))
    ps_pv = ctx.enter_context(tc.tile_pool(name="ps_pv", bufs=4, space="PSUM"))

    t_cm = persist.tile([P, CB, N], F32, tag="t_cm")
    t_r = persist.tile([P, CB, N], BF16, tag="t_r")
    NSUB = N // 512

    # ---- x first (largest + on the critical path): 4 x [128, 2048] ----
    XW = 2048
    for s_ in range(N // XW):
        sl = slice(s_ * XW, (s_ + 1) * XW)
        nc.sync.dma_start(out=t_cm[:, 0, sl], in_=x_d[0:P, sl])
        nc.scalar.dma_start(out=t_cm[:, 1, sl], in_=x_d[P : 2 * P, sl])

    # ---- tiny constants ----
    ident = persist.tile([P, P], F32, tag="ident")
    nc.sync.dma_start(out=ident, in_=ident_d[:, :])
    ident_r = persist.tile([P, P], F32R, tag="ident_r")
    nc.vector.tensor_copy(out=ident_r, in_=ident)
    gsum = persist.tile([P, G // CB], F32, tag="gsum")
    nc.sync.dma_start(out=gsum, in_=gsum_d[:, :])
    gbc = persist.tile([G // CB, P], F32, tag="gbc")
    nc.sync.dma_start(out=gbc, in_=gbc_d[:, :])

    def col_tile(dram_vec, tag):
        t = persist.tile([P, CB], F32, tag=tag)
        nc.sync.dma_start(out=t, in_=dram_vec[:].rearrange("(b p) -> p b", p=P))
        return t

    gamma_col = col_tile(gamma_d, "gamma_col")
    beta_col = col_tile(beta_d, "beta_col")
    bq_col = col_tile(b_d["q"], "bq_col")
    bk_col = col_tile(b_d["k"], "bk_col")
    bv_col = col_tile(b_d["v"], "bv_col")
    bp_col = col_tile(b_d["p"], "bp_col")

    # pre-scaled projection biases: sqrt(A) * b
    bqA_col = persist.tile([P, CB], F32, tag="bqA_col")
    nc.vector.tensor_scalar_mul(out=bqA_col, in0=bq_col, scalar1=SQA)
    bkA_col = persist.tile([P, CB], F32, tag="bkA_col")
    nc.vector.tensor_scalar_mul(out=bkA_col, in0=bk_col, scalar1=SQA)

    ebias_col = persist.tile([P, 1], F32, tag="ebias_col")
    nc.vector.memset(ebias_col, EBIAS)

    # ---- weights: stage f32, transpose (f32r) -> bf16 wT ----
    wT = {}
    w_stage = {}
    for nm in ("q", "k", "p", "v"):
        w_sb = staging.tile([P, CB, C], F32, tag="w_stage", name=f"w_sb_{nm}")
        nc.sync.dma_start(
            out=w_sb, in_=w_d[nm][:, :].rearrange("(b p) i -> p b i", p=P)
        )
        w_stage[nm] = w_sb
        if nm == "v":
            continue
        wt = persist.tile([P, CB, C], BF16, tag=f"w{nm}T")
        for b1 in range(CB):  # c_out block
            for b2 in range(CB):  # c_in block
                tp = ps_sc.tile([P, P], F32, tag="ps_sc", name=f"wtp_{nm}_{b1}_{b2}")
                nc.tensor.transpose(
                    tp, w_sb[:, b1, b2 * P : (b2 + 1) * P], ident
                )
                eng = nc.scalar if (b1 + b2) % 2 else nc.vector
                if eng is nc.scalar:
                    nc.scalar.copy(out=wt[:, b2, b1 * P : (b1 + 1) * P], in_=tp)
                else:
                    nc.vector.tensor_copy(out=wt[:, b2, b1 * P : (b1 + 1) * P], in_=tp)
        wT[nm] = wt

    # ---- WvpT = (Wp @ Wv).T in bf16: lhsT = Wv (natural) @ rhs WpT ----
    wv_r = persist.tile([P, CB, C], BF16, tag="wv_r")
    nc.scalar.copy(out=wv_r, in_=w_stage["v"])
    wvpT = persist.tile([P, CB, C], BF16, tag="wvpT")
    for ci_b in range(CB):
        pvp = ps_sc.tile([P, C], F32, tag="ps_sc", name=f"pvp_{ci_b}")
        for cm_b in range(CB):
            nc.tensor.matmul(
                pvp,
                lhsT=wv_r[:, cm_b, ci_b * P : (ci_b + 1) * P],
                rhs=wT["p"][:, cm_b, :],
                start=(cm_b == 0),
                stop=(cm_b == CB - 1),
            )
        nc.vector.tensor_copy(out=wvpT[:, ci_b, :], in_=pvp)

    # ---- bv2_col = (Wp @ bv) as per-channel column [P, CB] ----
    bv_bf = persist.tile([P, CB], BF16, tag="bv_bf")
    nc.vector.tensor_copy(out=bv_bf, in_=bv_col)
    bv2_col = persist.tile([P, CB], F32, tag="bv2_col")
    for cb in range(CB):
        bps = ps_pv.tile([P, 1], F32, tag="ps_pv", name=f"bv2ps_{cb}")
        for cm_b in range(CB):
            nc.tensor.matmul(
                bps,
                lhsT=wT["p"][:, cm_b, cb * P : (cb + 1) * P],
                rhs=bv_bf[:, cm_b : cm_b + 1],
                start=(cm_b == 0),
                stop=(cm_b == CB - 1),
            )
        nc.vector.tensor_copy(out=bv2_col[:, cb : cb + 1], in_=bps)

    # ---- GroupNorm stats ----
    gn_cols = []
    for cb in range(CB):
        xt = t_cm[:, cb, :]
        stats = stmp.tile([P, NSUB, 6], F32, tag="gn_stats")
        for s in range(NSUB):
            nc.vector.bn_stats(out=stats[:, s, :], in_=xt[:, s * 512 : (s + 1) * 512])
        mv = stmp.tile([P, 2], F32, tag="gn_mv")
        nc.vector.bn_aggr(out=mv, in_=stats)
        stats2 = stmp.tile([P, 2], F32, tag="gn_stats2")
        nc.vector.tensor_copy(out=stats2[:, 0:1], in_=mv[:, 0:1])
        nc.vector.tensor_tensor(
            out=stats2[:, 1:2], in0=mv[:, 0:1], in1=mv[:, 0:1], op=OP.mult
        )
        nc.vector.tensor_add(out=stats2[:, 1:2], in0=stats2[:, 1:2], in1=mv[:, 1:2])
        gps = ps_pv.tile([G // CB, 2], F32, tag="ps_pv", name=f"gps_{cb}")
        nc.tensor.matmul(gps, lhsT=gsum, rhs=stats2, start=True, stop=True)
        gsb = stmp.tile([G // CB, 2], F32, tag="gn_gsb")
        nc.vector.tensor_copy(out=gsb, in_=gps)
        gpack = stmp.tile([G // CB, 2], F32, tag="gn_gpack")
        nc.vector.tensor_copy(out=gpack[:, 0:1], in_=gsb[:, 0:1])
        gvar = stmp.tile([G // CB, 1], F32, tag="gn_gvar")
        nc.vector.tensor_tensor(out=gvar, in0=gsb[:, 0:1], in1=gsb[:, 0:1], op=OP.mult)
        nc.vector.tensor_tensor(out=gvar, in0=gsb[:, 1:2], in1=gvar, op=OP.subtract)
        eps_t = stmp.tile([G // CB, 1], F32, tag="gn_eps")
        nc.vector.memset(eps_t, EPS)
        nc.scalar.activation(out=gvar, in_=gvar, func=AF.Sqrt, bias=eps_t)
        nc.vector.reciprocal(out=gpack[:, 1:2], in_=gvar)
        bps = ps_pv.tile([P, 2], F32, tag="ps_pv", name=f"gbps_{cb}")
        nc.tensor.matmul(bps, lhsT=gbc, rhs=gpack, start=True, stop=True)
        s_col = stmp.tile([P, 1], F32, tag="gn_scol")
        nc.vector.tensor_tensor(
            out=s_col, in0=bps[:, 1:2], in1=gamma_col[:, cb : cb + 1], op=OP.mult
        )
        b_col = stmp.tile([P, 1], F32, tag="gn_bcol")
        nc.vector.tensor_tensor(out=b_col, in0=bps[:, 0:1], in1=s_col, op=OP.mult)
        nc.vector.tensor_tensor(
            out=b_col, in0=beta_col[:, cb : cb + 1], in1=b_col, op=OP.subtract
        )
        # residual copy gets + bp + Wp@bv folded in
        bpb_col = stmp.tile([P, 1], F32, tag="gn_bpb")
        nc.vector.tensor_add(out=bpb_col, in0=b_col, in1=bp_col[:, cb : cb + 1])
        nc.vector.tensor_add(out=bpb_col, in0=bpb_col, in1=bv2_col[:, cb : cb + 1])
        gn_cols.append((s_col, b_col, bpb_col))

    # ---- t_r (bf16 normalized tokens): DVE cb0 / ACT cb1 early, Pool late
    for sch in range(NSUB):
        asl = slice(sch * 512, (sch + 1) * 512)
        s0, b0, _ = gn_cols[0]
        s1, b1, _ = gn_cols[1]
        if sch < 5:
            nc.vector.tensor_scalar(
                out=t_r[:, 0, asl], in0=t_cm[:, 0, asl], scalar1=s0,
                scalar2=b0, op0=OP.mult, op1=OP.add,
            )
            nc.scalar.activation(
                out=t_r[:, 1, asl], in_=t_cm[:, 1, asl], func=AF.Identity,
                bias=b1, scale=s1,
            )
        else:
            nc.gpsimd.tensor_scalar(
                out=t_r[:, 0, asl], in0=t_cm[:, 0, asl], scalar1=s0,
                scalar2=b0, op0=OP.mult, op1=OP.add,
            )
            nc.gpsimd.tensor_scalar(
                out=t_r[:, 1, asl], in0=t_cm[:, 1, asl], scalar1=s1,
                scalar2=b1, op0=OP.mult, op1=OP.add,
            )

    # fp32 residual apply on Pool: t_cm := x*s + (b + bp + bv2)
    def apply_chunk(ch):
        asl = slice(ch * 512, (ch + 1) * 512)
        for cb in range(CB):
            s_col, _, bpb_col = gn_cols[cb]
            nc.gpsimd.tensor_scalar(
                out=t_cm[:, cb, asl], in0=t_cm[:, cb, asl], scalar1=s_col,
                scalar2=bpb_col, op0=OP.mult, op1=OP.add,
            )

    # ---- Q, K (fp8, pre-scaled), V2 (fp8 + ones col) ----
    q_cm = persist.tile([P, CB, N], FP8, tag="q_cm")
    k_cm = persist.tile([P, CB, N], FP8, tag="k_cm")
    v_aug = persist.tile([P, MB, VW], FP8, tag="v_aug")
    nc.vector.memset(v_aug[:, :, DCOL:VW], 0.0)
    nc.vector.memset(v_aug[:, :, DCOL : DCOL + 1], 1.0)

    def qk_chunk(nm, ch, dve):
        """project tokens [ch*512:(ch+1)*512] -> q_cm/k_cm fp8."""
        sl = slice(ch * NCHUNK, (ch + 1) * NCHUNK)
        wt = wT[nm]
        dst = q_cm if nm == "q" else k_cm
        bias = bqA_col if nm == "q" else bkA_col
        pq = ps_sc.tile([P, CB, NCHUNK], F32, tag="ps_sc", name=f"p{nm}_{ch}")
        for cb in range(CB):
            for ci in range(CB):
                nc.tensor.matmul(
                    pq[:, cb, :],
                    lhsT=wt[:, ci, cb * P : (cb + 1) * P],
                    rhs=t_r[:, ci, sl],
                    start=(ci == 0),
                    stop=(ci == CB - 1),
                )
        for cb in range(CB):
            if dve:
                nc.vector.tensor_scalar(
                    out=dst[:, cb, sl], in0=pq[:, cb, :], scalar1=SQA,
                    scalar2=bias[:, cb : cb + 1], op0=OP.mult, op1=OP.add,
                )
            else:
                nc.scalar.activation(
                    out=dst[:, cb, sl], in_=pq[:, cb, :], func=AF.Identity,
                    scale=SQA, bias=bias[:, cb : cb + 1],
                )

    def v_pair(pp, dve):
        """v2 for key blocks 2pp, 2pp+1 -> v_aug."""
        pv = ps_sc.tile([P, 2, C], F32, tag="ps_sc", name=f"pv_{pp}")
        for h in range(2):
            nb = 2 * pp + h
            for ci in range(CB):
                nc.tensor.matmul(
                    pv[:, h, :],
                    lhsT=t_r[:, ci, nb * P : (nb + 1) * P],
                    rhs=wvpT[:, ci, :],
                    start=(ci == 0),
                    stop=(ci == CB - 1),
                )
        dst = v_aug[:, 2 * pp : 2 * pp + 2, 0:C]
        if dve:
            nc.vector.tensor_copy(out=dst, in_=pv)
        else:
            nc.scalar.copy(out=dst, in_=pv)

    # ---- attention ----
    def scores_pair(j, pp):
        jsl = slice((j % NJ) * NCHUNK, (j % NJ + 1) * NCHUNK)
        sc = ps_sc.tile([P, 2, NCHUNK], F32, tag="ps_sc", name=f"sc_{j}_{pp}")
        for h in range(2):
            i = 2 * pp + h
            nc.tensor.matmul(
                sc[:, h, :],
                lhsT=k_cm[:, :, i * P : (i + 1) * P],
                rhs=q_cm[:, :, jsl],
                start=True,
                stop=True,
                perf_mode=DR,
            )
        ex = sexp.tile([P, 2, NCHUNK], FP8, tag="exp")
        if act_pairs[pp]:
            nc.scalar.activation(
                out=ex, in_=sc, func=AF.Exp, scale=LN2_8, bias=ebias_col
            )
        else:
            nc.vector.tensor_scalar(
                out=ex.bitcast(U8), in0=sc, scalar1=BS, scalar2=0.0,
                op0=OP.add, op1=OP.max,
            )
        return ex

    def pv_pair(pv_ps, pp, ex):
        for jj in range(JJ):
            nc.tensor.matmul(
                pv_ps[jj],
                lhsT=ex[:, :, jj * P : (jj + 1) * P],
                rhs=v_aug[:, 2 * pp : 2 * pp + 2, :],
                start=(pp == 0),
                stop=(pp == NP - 1),
                perf_mode=DR,
            )

    def pv_tiles(j):
        return [
            ps_pv.tile([P, VW], F32, tag="ps_pv", name=f"pv_ps_{j}_{jj}")
            for jj in range(JJ)
        ]

    def epilogue(j, pv_ps):
        jn = j % NJ
        jsl = slice(jn * NCHUNK, (jn + 1) * NCHUNK)
        anm = stmp.tile([P, JJ, C], F32R, tag="anm", name=f"anm_{j}")
        for jj in range(JJ):
            rec = stmp.tile([P, 1], F32, tag="rec")
            nc.vector.reciprocal(out=rec, in_=pv_ps[jj][:, DCOL : DCOL + 1])
            nc.scalar.activation(
                out=anm[:, jj, :], in_=pv_ps[jj][:, 0:C], func=AF.Copy,
                scale=rec, bias=0.0,
            )
        obs = sout.tile([P, CB, NCHUNK], F32, tag="out", name=f"ob_{j}")
        for co in range(CB):
            tp = ps_pv.tile([P, NCHUNK], F32R, tag="ps_pv", name=f"tp_{j}_{co}")
            for jj in range(JJ):
                nc.tensor.transpose(
                    tp[:, jj * P : (jj + 1) * P],
                    anm[:, jj, co * P : (co + 1) * P],
                    ident_r,
                )
            nc.vector.tensor_add(
                out=obs[:, co, :], in0=tp, in1=t_cm[:, co, jsl]
            )
        nc.sync.dma_start(
            out=out_d[:, jsl].rearrange("(b p) n -> p b n", p=P), in_=obs
        )

    # ---- chunk 0, interleaved with K/V production ----
    qk_chunk("q", 0, dve=True)
    pv_ps0 = pv_tiles(0)
    exs = {}
    for ch in range(NJ):
        qk_chunk("k", ch, dve=False)
        v_pair(2 * ch, dve=True)
        v_pair(2 * ch + 1, dve=False)
        for pp in (2 * ch, 2 * ch + 1):
            exs[pp] = scores_pair(0, pp)
            if pp >= 1:
                pv_pair(pv_ps0, pp - 1, exs.pop(pp - 1))
    pv_pair(pv_ps0, NP - 1, exs.pop(NP - 1))
    apply_chunk(0)
    qk_chunk("q", 1, dve=True)
    epilogue(0, pv_ps0)

    # ---- remaining chunks ----
    for j in range(1, NJ * att_reps):
        if j < NJ:
            apply_chunk(j)
        pv_ps = pv_tiles(j)
        exs = {0: scores_pair(j, 0), 1: scores_pair(j, 1)}
        for pp in range(NP):
            if pp + 2 < NP:
                exs[pp + 2] = scores_pair(j, pp + 2)
            if pp == 10 and j + 1 < NJ:
                qk_chunk("q", j + 1, dve=True)
            pv_pair(pv_ps, pp, exs.pop(pp))
        epilogue(j, pv_ps)


def kernel(x, gamma, beta, Wq, bq, Wk, bk, Wv, bv, Wp, bp):
    if "nc" not in _CACHE:
        _CACHE["nc"] = build_nc()
    nc = _CACHE["nc"]

    x = np.ascontiguousarray(np.asarray(x, dtype=np.float32)).reshape(B, C, N)
    common = {
        "gamma": np.asarray(gamma, np.float32),
        "beta": np.asarray(beta, np.float32),
        "Wq": np.asarray(Wq, np.float32),
        "bq": np.asarray(bq, np.float32),
        "Wk": np.asarray(Wk, np.float32),
        "bk": np.asarray(bk, np.float32),
        "Wv": np.asarray(Wv, np.float32),
        "bv": np.asarray(bv, np.float32),
        "Wp": np.asarray(Wp, np.float32),
        "bp": np.asarray(bp, np.float32),
    }
    in_maps = [{"x": x[b], **common} for b in range(B)]
    res = bass_utils.run_bass_kernel_spmd(nc, in_maps, core_ids=list(range(B)))
    out = np.stack([res.results[b]["out"] for b in range(B)])
    return out.reshape(B, C, H, W)


# revision 54
# speedup vs baseline: 24.1034x; 24.1034x over previous
"""Trainium2 Bass kernel for an AttentionBlock (GroupNorm + single-head
full N^2 attention + output projection + residual), data-parallel over
batch: 8 samples on 8 NeuronCores, no collectives.

Shapes (hardcoded): x [8, 256, 64, 64]; weights [256, 256]; biases [256].
Per core: one batch sample, x viewed as [C=256, N=4096] channel-major.

Per-core pipeline (fp8e4m3 DoubleRow attention matmuls, bf16 projections):
  1. GroupNorm (8 groups) in C-major layout: per-partition bn_stats,
     cross-partition group reduction via tiny constant matmuls, applied as
     per-partition scale/bias. Rounded tokens t_r are bf16; the fp32
     residual copy (with bp + Wp@bv folded in) runs on the Pool engine,
     overlapped with attention.
  2. Q,K projections (bf16) emit fp8 q',k' pre-scaled by sqrt(A) with
     A = 8*log2(e)/sqrt(C), so the QK^T PSUM value is directly the fp8
     exponent-byte coordinate: byte = psum + BS. The output projection is
     folded into V: v2 = t @ (Wp Wv).T in bf16 -> fp8, with a ones column
     so PV also produces softmax denominators; Wp@bv moves to the final
     residual bias.
  3. Attention per 512-query chunk over 16 key-block *pairs*: one fp8
     DoubleRow matmul per key block (contraction 256 in one instruction,
     0.5 cyc/row); softmax exp is split across engines per pair:
       ACT: out = Exp(ln2/8 * psum + bias) written as fp8e4m3
       DVE: byte = round(max(psum + BS, 0)) as uint8 == the fp8 bit
            pattern of the same alpha*exp(s) (Schraudolph-in-fp8)
     Both paths share the scale alpha = 2^((BS-56)/8), which cancels in
     the softmax normalization. PV accumulates [proj | colsum] in PSUM
     with fp8 DoubleRow over pairs.
  4. Epilogue per chunk: per-128-query reciprocal + ACT copy-scale
     normalize, f32r TensorE transposes back to C-major, DVE residual add
     against the Pool-prepared t_cm, single DMA out per chunk.
"""

import numpy as np

import concourse.bacc as bacc
import concourse.mybir as mybir
import concourse.tile as tile
from concourse import bass_utils

F32 = mybir.dt.float32
F32R = mybir.dt.float32r
BF16 = mybir.dt.bfloat16
FP8 = mybir.dt.float8e4
U8 = mybir.dt.uint8
AF = mybir.ActivationFunctionType
OP = mybir.AluOpType
DR = mybir.MatmulPerfMode.DoubleRow

B = 8
C = 256
H = 64
W = 64
N = H * W  # 4096 tokens
G = 8  # groups
GS = C // G  # 32 channels per group
P = 128
CB = C // P  # 2 channel blocks
EPS = 1e-5
NCHUNK = 512  # query chunk
NJ = N // NCHUNK  # 8
MB = N // P  # 32 key blocks
NP = MB // 2  # 16 key-block pairs
JJ = NCHUNK // P  # 4 query sub-blocks per chunk
VW = 272  # padded v_aug width (DoubleRow pair stride must be %16)
DCOL = 256  # denominator (ones) column index

SQA = float(np.sqrt(8.0 * np.log2(np.e) / 16.0))  # 0.849329
BS = 20.0  # byte bias: byte = psum + BS
LN2_8 = float(np.log(2.0) / 8.0)
EBIAS = float((BS - 56.0) * np.log(2.0) / 8.0)

# exp engine assignment per pair: True -> ACT, False -> DVE
ACT_PAIRS = tuple(p % 2 == 0 for p in range(NP))

_CACHE: dict = {}


def build_nc(att_reps=1, act_pairs=ACT_PAIRS):
    nc = bacc.Bacc(
        "TRN2",
        target_bir_lowering=False,
        debug=False,
        enable_asserts=False,
        num_devices=B,
    )

    x_d = nc.dram_tensor("x", [C, N], F32, kind="ExternalInput")
    gamma_d = nc.dram_tensor("gamma", [C], F32, kind="ExternalInput")
    beta_d = nc.dram_tensor("beta", [C], F32, kind="ExternalInput")
    w_d = {}
    b_d = {}
    for nm in ("q", "k", "v", "p"):
        w_d[nm] = nc.dram_tensor(f"W{nm}", [C, C], F32, kind="ExternalInput")
        b_d[nm] = nc.dram_tensor(f"b{nm}", [C], F32, kind="ExternalInput")
    out_d = nc.dram_tensor("out", [C, N], F32, kind="ExternalOutput")

    import ml_dtypes

    ident_d = nc.inline_tensor(np.eye(P, dtype=np.float32), name="ident")
    identbf_d = nc.inline_tensor(
        np.eye(P, dtype=ml_dtypes.bfloat16), name="identbf"
    )
    gsum_np = np.zeros((P, G // CB), np.float32)
    for p in range(P):
        gsum_np[p, p // GS] = 1.0 / GS
    gsum_d = nc.inline_tensor(gsum_np, name="gsum")
    gbc_np = np.zeros((G // CB, P), np.float32)
    for p in range(P):
        gbc_np[p // GS, p] = 1.0
    gbc_d = nc.inline_tensor(gbc_np, name="gbc")

    from contextlib import ExitStack

    with tile.TileContext(nc) as tc:
        with ExitStack() as ctx:
            _build_tile(
                ctx, tc, x_d, gamma_d, beta_d, w_d, b_d, out_d, ident_d,
                identbf_d, gsum_d, gbc_d, att_reps, act_pairs,
            )
    nc.compile()
    return nc


def _build_tile(ctx, tc, x_d, gamma_d, beta_d, w_d, b_d, out_d, ident_d,
                identbf_d, gsum_d, gbc_d, att_reps, act_pairs):
    nc = tc.nc

    persist = ctx.enter_context(tc.tile_pool(name="persist", bufs=1))
    staging = ctx.enter_context(tc.tile_pool(name="staging", bufs=4))
    sexp = ctx.enter_context(tc.tile_pool(name="sexp", bufs=36))
    sout = ctx.enter_context(tc.tile_pool(name="sout", bufs=2))
    sanm = ctx.enter_context(tc.tile_pool(name="sanm", bufs=2))
    stmp = ctx.enter_context(tc.tile_pool(name="stmp", bufs=6))
    ps_sc = ctx.enter_context(tc.tile_pool(name="ps_sc", bufs=3, space="PSUM"))
    ps_pv = ctx.enter_context(tc.tile_pool(name="ps_pv", bufs=2, space="PSUM"))

    t_cm = persist.tile([P, CB, N], BF16, tag="t_cm")
    t_r = persist.tile([P, CB, N], BF16, tag="t_r")
    NSUB = N // 512

    # ---- x first (critical path): casting SWDGE DMAs f32 -> bf16 (halves
    # the modeled DMA-bus time; HWDGE stays free so the tiny constants land
    # on the bus first). 8 pieces so bn_stats starts on the first KB.
    XW = 2048
    for q in range(N // XW):
        sl = slice(q * XW, (q + 1) * XW)
        for cb in range(CB):
            nc.gpsimd.dma_start(
                out=t_cm[:, cb, sl], in_=x_d[cb * P : (cb + 1) * P, sl]
            )

    # ---- weights: casting DMAs to bf16 staging ----
    w_dma = {}
    for nm in ("q", "k", "p", "v"):
        w_sb = staging.tile([P, CB, C], BF16, tag="w_stage", name=f"w_sb_{nm}")
        nc.gpsimd.dma_start(
            out=w_sb, in_=w_d[nm][:, :].rearrange("(b p) i -> p b i", p=P)
        )
        w_dma[nm] = w_sb

    # ---- tiny constants on HWDGE (transfer ahead of the big loads) ----
    gsum = persist.tile([P, G // CB], F32, tag="gsum")
    nc.sync.dma_start(out=gsum, in_=gsum_d[:, :])
    gbc = persist.tile([G // CB, P], F32, tag="gbc")
    nc.sync.dma_start(out=gbc, in_=gbc_d[:, :])

    def col_tile(dram_vec, tag, q=nc.sync):
        t = persist.tile([P, CB], F32, tag=tag)
        q.dma_start(out=t, in_=dram_vec[:].rearrange("(b p) -> p b", p=P))
        return t

    gamma_col = col_tile(gamma_d, "gamma_col")
    beta_col = col_tile(beta_d, "beta_col")
    # NOTE: bk is dropped entirely -- adding bk to K shifts each query's
    # scores by a per-query constant along the key axis, which softmax
    # cancels exactly.
    bq_row = persist.tile([1, C], F32, tag="bq_row")
    nc.scalar.dma_start(
        out=bq_row, in_=b_d["q"][:].rearrange("(a c) -> a c", a=1)
    )
    ident = persist.tile([P, P], F32, tag="ident")
    nc.sync.dma_start(out=ident, in_=ident_d[:, :])
    ident_bf = persist.tile([P, P], BF16, tag="ident_bf")
    nc.scalar.dma_start(out=ident_bf, in_=identbf_d[:, :])
    ident_r = persist.tile([P, P], F32R, tag="ident_r")
    nc.scalar.copy(out=ident_r, in_=ident)
    bv_col = col_tile(b_d["v"], "bv_col", q=nc.scalar)
    bp_col = col_tile(b_d["p"], "bp_col", q=nc.scalar)

    # q-bias as a bf16 row (folded into the q PSUM via a rank-1 matmul)
    bq_bf = persist.tile([1, C], BF16, tag="bq_bf")
    nc.gpsimd.tensor_copy(out=bq_bf, in_=bq_row)
    ones512_bf = persist.tile([1, NCHUNK], BF16, tag="ones512_bf")
    nc.gpsimd.memset(ones512_bf, 1.0)

    ebias_col = persist.tile([P, 1], F32, tag="ebias_col")
    nc.gpsimd.memset(ebias_col, EBIAS)

    # ---- GN stats first in the DVE queue (don't block behind consts) ----
    all_stats = []
    for cb in range(CB):
        stats = stmp.tile([P, NSUB, 6], F32, tag="gn_stats", name=f"stats_{cb}")
        all_stats.append(stats)
    # piece order matches the x DMA arrival order
    for q in range(N // XW):
        for cb in range(CB):
            for s in range(4 * q, 4 * q + 4):
                nc.vector.bn_stats(
                    out=all_stats[cb][:, s, :],
                    in_=t_cm[:, cb, s * 512 : (s + 1) * 512],
                )

    # ---- weights: bf16 transpose via regular matmul against bf16 ident:
    # out = w_slice.T @ I. Copies PSUM -> bf16 SBUF on ACT (idle early).
    wT = {}
    w_stage = w_dma
    for nm in ("q", "k", "p"):
        w_sb = w_stage[nm]
        wt = persist.tile([P, CB, C], BF16, tag=f"w{nm}T")
        for b1 in range(CB):  # c_out block
            for b2 in range(CB):  # c_in block
                tp = ps_sc.tile([P, P], F32, tag="ps_sc", name=f"wtp_{nm}_{b1}_{b2}")
                nc.tensor.matmul(
                    tp, lhsT=w_sb[:, b1, b2 * P : (b2 + 1) * P], rhs=ident_bf,
                    start=True, stop=True,
                )
                nc.scalar.copy(out=wt[:, b2, b1 * P : (b1 + 1) * P], in_=tp)
        wT[nm] = wt

    # ---- WvpT = (Wp @ Wv).T in bf16: lhsT = Wv (natural) @ rhs WpT ----
    wv_r = w_stage["v"]
    wvpT = persist.tile([P, CB, C], BF16, tag="wvpT")
    for ci_b in range(CB):
        pvp = ps_sc.tile([P, C], F32, tag="ps_sc", name=f"pvp_{ci_b}")
        for cm_b in range(CB):
            nc.tensor.matmul(
                pvp,
                lhsT=wv_r[:, cm_b, ci_b * P : (ci_b + 1) * P],
                rhs=wT["p"][:, cm_b, :],
                start=(cm_b == 0),
                stop=(cm_b == CB - 1),
            )
        nc.scalar.copy(out=wvpT[:, ci_b, :], in_=pvp)

    # ---- bv2_col = (Wp @ bv) as per-channel column [P, CB] ----
    bv_bf = persist.tile([P, CB], BF16, tag="bv_bf")
    nc.scalar.copy(out=bv_bf, in_=bv_col)
    bv2_col = persist.tile([P, CB], F32, tag="bv2_col")
    for cb in range(CB):
        bps = ps_pv.tile([P, 1], F32, tag="ps_pv", name=f"bv2ps_{cb}")
        for cm_b in range(CB):
            nc.tensor.matmul(
                bps,
                lhsT=wT["p"][:, cm_b, cb * P : (cb + 1) * P],
                rhs=bv_bf[:, cm_b : cm_b + 1],
                start=(cm_b == 0),
                stop=(cm_b == CB - 1),
            )
        nc.vector.tensor_copy(out=bv2_col[:, cb : cb + 1], in_=bps)

    # ---- GroupNorm reduction ----
    gn_cols = []
    for cb in range(CB):
        stats = all_stats[cb]
        mv = stmp.tile([P, 2], F32, tag="gn_mv")
        nc.vector.bn_aggr(out=mv, in_=stats)
        stats2 = stmp.tile([P, 2], F32, tag="gn_stats2")
        nc.vector.tensor_copy(out=stats2[:, 0:1], in_=mv[:, 0:1])
        nc.vector.tensor_tensor(
            out=stats2[:, 1:2], in0=mv[:, 0:1], in1=mv[:, 0:1], op=OP.mult
        )
        nc.vector.tensor_add(out=stats2[:, 1:2], in0=stats2[:, 1:2], in1=mv[:, 1:2])
        gps = ps_pv.tile([G // CB, 2], F32, tag="ps_pv", name=f"gps_{cb}")
        nc.tensor.matmul(gps, lhsT=gsum, rhs=stats2, start=True, stop=True)
        gsb = stmp.tile([G // CB, 2], F32, tag="gn_gsb")
        nc.vector.tensor_copy(out=gsb, in_=gps)
        gpack = stmp.tile([G // CB, 2], F32, tag="gn_gpack")
        nc.vector.tensor_copy(out=gpack[:, 0:1], in_=gsb[:, 0:1])
        gvar = stmp.tile([G // CB, 1], F32, tag="gn_gvar")
        nc.vector.tensor_tensor(out=gvar, in0=gsb[:, 0:1], in1=gsb[:, 0:1], op=OP.mult)
        nc.vector.tensor_tensor(out=gvar, in0=gsb[:, 1:2], in1=gvar, op=OP.subtract)
        nc.vector.tensor_scalar_add(out=gvar, in0=gvar, scalar1=EPS)
        # rstd = rsqrt(gvar) fully on DVE (bit-trick seed + 3 Newton steps)
        # to keep the ACT Sqrt table load off the critical path.
        y = stmp.tile([G // CB, 1], F32, tag="gn_y")
        nc.vector.tensor_scalar(
            out=y.bitcast(mybir.dt.int32), in0=gvar.bitcast(mybir.dt.int32),
            scalar1=1, scalar2=None, op0=OP.logical_shift_right, op1=OP.bypass,
        )
        nc.vector.tensor_scalar(
            out=y.bitcast(mybir.dt.int32), in0=y.bitcast(mybir.dt.int32),
            scalar1=-1, scalar2=0x5F3759DF, op0=OP.mult, op1=OP.add,
        )
        vh = stmp.tile([G // CB, 1], F32, tag="gn_vh")
        nc.vector.tensor_scalar_mul(out=vh, in0=gvar, scalar1=-0.5)
        for _ in range(3):
            yy = stmp.tile([G // CB, 1], F32, tag="gn_yy")
            nc.vector.tensor_tensor(out=yy, in0=y, in1=y, op=OP.mult)
            nc.vector.tensor_tensor(out=yy, in0=yy, in1=vh, op=OP.mult)
            nc.vector.tensor_scalar_add(out=yy, in0=yy, scalar1=1.5)
            nc.vector.tensor_tensor(out=y, in0=y, in1=yy, op=OP.mult)
        nc.vector.tensor_copy(out=gpack[:, 1:2], in_=y)
        bps = ps_pv.tile([P, 2], F32, tag="ps_pv", name=f"gbps_{cb}")
        nc.tensor.matmul(bps, lhsT=gbc, rhs=gpack, start=True, stop=True)
        s_col = stmp.tile([P, 1], F32, tag="gn_scol")
        nc.vector.tensor_tensor(
            out=s_col, in0=bps[:, 1:2], in1=gamma_col[:, cb : cb + 1], op=OP.mult
        )
        b_col = stmp.tile([P, 1], F32, tag="gn_bcol")
        nc.vector.tensor_tensor(out=b_col, in0=bps[:, 0:1], in1=s_col, op=OP.mult)
        nc.vector.tensor_tensor(
            out=b_col, in0=beta_col[:, cb : cb + 1], in1=b_col, op=OP.subtract
        )
        # residual copy gets + bp + Wp@bv folded in
        bpb_col = stmp.tile([P, 1], F32, tag="gn_bpb")
        nc.vector.tensor_add(out=bpb_col, in0=b_col, in1=bp_col[:, cb : cb + 1])
        nc.vector.tensor_add(out=bpb_col, in0=bpb_col, in1=bv2_col[:, cb : cb + 1])
        gn_cols.append((s_col, b_col, bpb_col))

    # ---- t_r (bf16 normalized tokens): DVE cb0 / ACT cb1 early, Pool late
    s0, b0, _ = gn_cols[0]
    s1, b1, _ = gn_cols[1]
    for u in range(2):  # tokens [0:1024], [1024:2048] on DVE (cb0)/ACT (cb1)
        asl = slice(u * 1024, (u + 1) * 1024)
        nc.vector.tensor_scalar(
            out=t_r[:, 0, asl], in0=t_cm[:, 0, asl], scalar1=s0,
            scalar2=b0, op0=OP.mult, op1=OP.add,
        )
        nc.scalar.activation(
            out=t_r[:, 1, asl], in_=t_cm[:, 1, asl], func=AF.Identity,
            bias=b1, scale=s1,
        )
    for u in range(2, 4):  # tokens [2048:4096] both cb on Pool
        asl = slice(u * 1024, (u + 1) * 1024)
        for cb, (sc_, bc_) in enumerate(((s0, b0), (s1, b1))):
            nc.gpsimd.tensor_scalar(
                out=t_r[:, cb, asl], in0=t_cm[:, cb, asl], scalar1=sc_,
                scalar2=bc_, op0=OP.mult, op1=OP.add,
            )

    # fp32 residual apply on Pool: t_cm := x*s + (b + bp + bv2)
    def apply_chunk(ch):
        asl = slice(ch * 512, (ch + 1) * 512)
        for cb in range(CB):
            s_col, _, bpb_col = gn_cols[cb]
            nc.gpsimd.tensor_scalar(
                out=t_cm[:, cb, asl], in0=t_cm[:, cb, asl], scalar1=s_col,
                scalar2=bpb_col, op0=OP.mult, op1=OP.add,
            )

    # ---- Q, K (fp8, pre-scaled), V2 (fp8 + ones col) ----
    q_cm = persist.tile([P, CB, N], FP8, tag="q_cm")
    k_cm = persist.tile([P, CB, N], FP8, tag="k_cm")
    v_aug = persist.tile([P, MB, VW], FP8, tag="v_aug")
    nc.gpsimd.memset(v_aug[:, :, DCOL:VW], 0.0)
    nc.gpsimd.memset(v_aug[:, :, DCOL : DCOL + 1], 1.0)

    def qk_chunk(nm, ch, dve):
        """project tokens [ch*512:(ch+1)*512] -> q_cm/k_cm fp8.

        k drops its bias (softmax-invariant); q folds bq into the PSUM via
        a rank-1 matmul. The epilogue is then a single uniform-scale
        convert over both channel blocks."""
        sl = slice(ch * NCHUNK, (ch + 1) * NCHUNK)
        wt = wT[nm]
        dst = q_cm if nm == "q" else k_cm
        pq = ps_sc.tile([P, CB, NCHUNK], F32, tag="ps_sc", name=f"p{nm}_{ch}")
        for cb in range(CB):
            last = CB - 1 if nm == "k" else CB
            for ci in range(CB):
                nc.tensor.matmul(
                    pq[:, cb, :],
                    lhsT=wt[:, ci, cb * P : (cb + 1) * P],
                    rhs=t_r[:, ci, sl],
                    start=(ci == 0),
                    stop=(ci == last),
                )
            if nm == "q":
                nc.tensor.matmul(
                    pq[:, cb, :],
                    lhsT=bq_bf[:, cb * P : (cb + 1) * P],
                    rhs=ones512_bf,
                    start=False,
                    stop=True,
                )
        if dve:
            nc.vector.tensor_scalar_mul(out=dst[:, :, sl], in0=pq, scalar1=SQA)
        else:
            nc.scalar.mul(out=dst[:, :, sl], in_=pq, mul=SQA)

    def v_group(g, dve):
        """v2 for key blocks 4g..4g+3 (pairs 2g, 2g+1) -> v_aug."""
        pv = ps_sc.tile([P, 4, C], F32, tag="ps_sc", name=f"pv_{g}")
        for h in range(4):
            nb = 4 * g + h
            for ci in range(CB):
                nc.tensor.matmul(
                    pv[:, h, :],
                    lhsT=t_r[:, ci, nb * P : (nb + 1) * P],
                    rhs=wvpT[:, ci, :],
                    start=(ci == 0),
                    stop=(ci == CB - 1),
                )
        dst = v_aug[:, 4 * g : 4 * g + 4, 0:C]
        if dve:
            nc.vector.tensor_copy(out=dst, in_=pv)
        else:
            nc.scalar.copy(out=dst, in_=pv)

    # ---- attention ----
    def scores_pair(j, pp):
        jsl = slice((j % NJ) * NCHUNK, (j % NJ + 1) * NCHUNK)
        sc = ps_sc.tile([P, 2, NCHUNK], F32, tag="ps_sc", name=f"sc_{j}_{pp}")
        for h in range(2):
            i = 2 * pp + h
            nc.tensor.matmul(
                sc[:, h, :],
                lhsT=k_cm[:, :, i * P : (i + 1) * P],
                rhs=q_cm[:, :, jsl],
                start=True,
                stop=True,
                perf_mode=DR,
            )
        ex = sexp.tile([P, 2, NCHUNK], FP8, tag="exp")
        if act_pairs[pp] or (j > 0 and pp == NP - 1):
            nc.scalar.activation(
                out=ex, in_=sc, func=AF.Exp, scale=LN2_8, bias=ebias_col
            )
        else:
            nc.vector.tensor_scalar(
                out=ex.bitcast(U8), in0=sc, scalar1=BS, scalar2=0.0,
                op0=OP.add, op1=OP.max,
            )
        return ex

    def pv_half(pvAB, jlo, pp, ex):
        """PV for query sub-blocks jlo, jlo+1 (one pass) over pair pp."""
        for u in range(2):
            jj = jlo + u
            nc.tensor.matmul(
                pvAB[u],
                lhsT=ex[:, :, jj * P : (jj + 1) * P],
                rhs=v_aug[:, 2 * pp : 2 * pp + 2, :],
                start=(pp == 0),
                stop=(pp == NP - 1),
                perf_mode=DR,
            )

    def pv_half_tiles(j, jlo):
        return [
            ps_pv.tile([P, VW], F32, tag="ps_pv", name=f"pv_ps_{j}_{jlo + u}")
            for u in range(2)
        ]

    def norm_half(j, anm, pvAB, jlo, dve=False):
        for u in range(2):
            jj = jlo + u
            rec = stmp.tile([P, 1], F32, tag="rec")
            nc.vector.reciprocal(out=rec, in_=pvAB[u][:, DCOL : DCOL + 1])
            if dve:
                nc.vector.tensor_scalar_mul(
                    out=anm[:, jj, :], in0=pvAB[u][:, 0:C], scalar1=rec
                )
            else:
                nc.scalar.activation(
                    out=anm[:, jj, :], in_=pvAB[u][:, 0:C], func=AF.Copy,
                    scale=rec, bias=0.0,
                )

    def tail_co(j, anm, co):
        """transpose + residual-add + (after co1) store for chunk j."""
        jn = j % NJ
        jsl = slice(jn * NCHUNK, (jn + 1) * NCHUNK)
        if co == 0:
            obs = sout.tile([P, CB, NCHUNK], F32, tag="out", name=f"ob_{j}")
            tail_obs[j] = obs
        else:
            obs = tail_obs[j]
        tp = ps_pv.tile([P, NCHUNK], F32R, tag="ps_pv", name=f"tp_{j}_{co}")
        for jj in range(JJ):
            nc.tensor.transpose(
                tp[:, jj * P : (jj + 1) * P],
                anm[:, jj, co * P : (co + 1) * P],
                ident_r,
            )
        nc.vector.tensor_add(out=obs[:, co, :], in0=tp, in1=t_cm[:, co, jsl])
        if co == CB - 1:
            nc.sync.dma_start(
                out=out_d[:, jsl].rearrange("(b p) n -> p b n", p=P), in_=obs
            )

    tail_obs = {}

    def chunk_tail(prev):
        """pass-2 PV + epilogue for the previous chunk, as one block (used
        for chunk 0's predecessor slot and the final flush)."""
        if prev is None:
            return
        j, exs, anm = prev
        pvCD = pv_half_tiles(j, 2)
        for pp in range(NP):
            pv_half(pvCD, 2, pp, exs[pp])
        norm_half(j, anm, pvCD, 2)
        tail_co(j, anm, 0)
        tail_co(j, anm, 1)

    prev = None  # (j, exs, anm) awaiting pass-2 + epilogue

    # ---- chunk 0: prime K/V for ch 0-1, then one K-or-V production per
    # scores slot (SBUF-resident k_cm/v_aug decouple the streams).
    qk_chunk("q", 0, dve=True)
    qk_chunk("k", 0, dve=False)
    v_group(0, dve=True)
    qk_chunk("k", 1, dve=True)
    v_group(1, dve=False)
    exs0 = {}
    pvAB = None
    for pp in range(NP):
        exs0[pp] = scores_pair(0, pp)
        if pp < 12:
            ch = 2 + pp // 2
            if pp % 2 == 0:
                qk_chunk("k", ch, dve=(ch % 2 == 1))
            else:
                v_group(ch, dve=(ch % 2 == 0))
        elif pp == 12:
            qk_chunk("q", 1, dve=False)
        if pp == 3:
            pvAB = pv_half_tiles(0, 0)
        if pp >= 3:
            pv_half(pvAB, 0, pp - 3, exs0[pp - 3])
    for pq_ in range(NP - 3, NP):
        pv_half(pvAB, 0, pq_, exs0[pq_])
    apply_chunk(0)
    prev = (0, exs0, None, pvAB)

    # ---- remaining chunks: the scores/exp stream for chunk j carries the
    # deferred pass-2 + epilogue of chunk j-1, interleaved finely so the
    # exp engines never wait behind bulk PE work in issue order.
    for j in range(1, NJ * att_reps):
        if j < NJ:
            apply_chunk(j)
        pj, pexs, _, ppvAB = prev
        exs = {0: scores_pair(j, 0)}
        # deferred pass-1 normalize of the previous chunk: emitted here so
        # the new chunk's score stream is never queued behind it
        panm = sanm.tile([P, JJ, C], F32R, tag="anm", name=f"anm_{pj}")
        norm_half(pj, panm, ppvAB, 0)
        pvCD = pv_half_tiles(pj, 2)
        pvAB = None
        for pp in range(NP):
            if pp > 0:
                exs[pp] = scores_pair(j, pp)
            if pp < 8:  # pass-2 of j-1, two pairs per slot
                pv_half(pvCD, 2, 2 * pp, pexs[2 * pp])
                pv_half(pvCD, 2, 2 * pp + 1, pexs[2 * pp + 1])
            elif pp == 10:
                if j + 1 < NJ:
                    qk_chunk("q", j + 1, dve=False)
            elif pp == 11:
                norm_half(pj, panm, pvCD, 2, dve=True)
                tail_co(pj, panm, 0)
            elif pp == 12:
                tail_co(pj, panm, 1)
            elif pp == 13:
                pvAB = pv_half_tiles(j, 0)
                pv_half(pvAB, 0, 0, exs[0])
                pv_half(pvAB, 0, 1, exs[1])
            elif pp >= 14:  # pairs 2..5 over slots 14,15
                lo = 2 * (pp - 13)
                pv_half(pvAB, 0, lo, exs[lo])
                pv_half(pvAB, 0, lo + 1, exs[lo + 1])
        for pq_ in range(6, NP):
            pv_half(pvAB, 0, pq_, exs[pq_])
        prev = (j, exs, panm, pvAB)

    j_l, exs_l, _, pvAB_l = prev
    anm_l = sanm.tile([P, JJ, C], F32R, tag="anm", name=f"anm_{j_l}")
    norm_half(j_l, anm_l, pvAB_l, 0)
    chunk_tail((j_l, exs_l, anm_l))


def kernel(x, gamma, beta, Wq, bq, Wk, bk, Wv, bv, Wp, bp):
    if "nc" not in _CACHE:
        _CACHE["nc"] = build_nc()
    nc = _CACHE["nc"]

    x = np.ascontiguousarray(np.asarray(x, dtype=np.float32)).reshape(B, C, N)
    common = {
        "gamma": np.asarray(gamma, np.float32),
        "beta": np.asarray(beta, np.float32),
        "Wq": np.asarray(Wq, np.float32),
        "bq": np.asarray(bq, np.float32),
        "Wk": np.asarray(Wk, np.float32),
        "bk": np.asarray(bk, np.float32),
        "Wv": np.asarray(Wv, np.float32),
        "bv": np.asarray(bv, np.float32),
        "Wp": np.asarray(Wp, np.float32),
        "bp": np.asarray(bp, np.float32),
    }
    in_maps = [{"x": x[b], **common} for b in range(B)]
    res = bass_utils.run_bass_kernel_spmd(nc, in_maps, core_ids=list(range(B)))
    out = np.stack([res.results[b]["out"] for b in range(B)])
    return out.reshape(B, C, H, W)


# revision 66
# speedup vs baseline: 24.2541x; 1.0063x over previous
"""Trainium2 Bass kernel for an AttentionBlock (GroupNorm + single-head
full N^2 attention + output projection + residual), data-parallel over
batch: 8 samples on 8 NeuronCores, no collectives.

Shapes (hardcoded): x [8, 256, 64, 64]; weights [256, 256]; biases [256].
Per core: one batch sample, x viewed as [C=256, N=4096] channel-major.

Per-core pipeline (fp8e4m3 DoubleRow attention matmuls, bf16 projections):
  1. x loads as bf16 via casting SWDGE DMAs (t_cm); GroupNorm stats run
     piecewise behind the load, with the cross-partition group reduction
     (tiny constant matmuls + an all-DVE bit-trick rsqrt) overlapped with
     the last stats batch. Normalized tokens t_r are bf16 (DVE 4x-mode
     tensor_scalar); the residual copy of t_cm (with bp + Wp@bv folded in)
     runs on the Pool engine under the attention phase.
  2. Q,K projections (bf16) emit fp8 q',k' pre-scaled by sqrt(A),
     A = 8*log2(e)/sqrt(C), so the QK^T PSUM value is directly the fp8
     exponent-byte coordinate: byte = psum + BS. bq folds into the q PSUM
     via a rank-1 matmul (epilogues are single uniform-scale converts);
     bk is dropped entirely (a per-query constant along keys -- softmax
     cancels it exactly). The output projection folds into V:
     v2 = t @ (Wp Wv).T with a ones column so PV also produces softmax
     denominators; Wp@bv moves to the final residual bias.
  3. Attention per 512-query chunk over 16 key-block *pairs*: one fp8
     DoubleRow matmul per key block (contraction 256 in one instruction,
     0.5 cyc/row) into [P,2,512] PSUM tiles (3-deep rotation); softmax exp
     is split across engines per pair:
       ACT: out = Exp(ln2/8 * psum + bias) written as fp8e4m3
       DVE: byte = round(max(psum + BS, 0)) as uint8 == the fp8 bit
            pattern of the same alpha*exp(s) (Schraudolph-in-fp8)
     Both paths share the scale alpha = 2^((BS-56)/8), which cancels in
     the softmax normalization. All 16 fp8 ex tiles are retained in SBUF;
     PV runs as two passes (query sub-blocks 0-1, then 2-3) so only two
     PSUM accumulator banks are live, and pass 2 + the epilogue of chunk
     j-1 are software-pipelined into chunk j's score/exp stream. Chunk 0
     interleaves one K-chunk or V-group production per score slot.
  4. Epilogue per chunk: per-128-query reciprocal + copy-scale normalize,
     f32r TensorE transposes back to C-major, DVE residual add against
     the Pool-prepared t_cm, single DMA out per chunk.

Cost-model total: ~145.7 us/core (baseline fp32r kernel: 293.3 us);
max rel err vs the fp32 reference: 1.26e-2 (gate 2e-2).
"""

import numpy as np

import concourse.bacc as bacc
import concourse.mybir as mybir
import concourse.tile as tile
from concourse import bass_utils

F32 = mybir.dt.float32
F32R = mybir.dt.float32r
BF16 = mybir.dt.bfloat16
FP8 = mybir.dt.float8e4
U8 = mybir.dt.uint8
AF = mybir.ActivationFunctionType
OP = mybir.AluOpType
DR = mybir.MatmulPerfMode.DoubleRow

B = 8
C = 256
H = 64
W = 64
N = H * W  # 4096 tokens
G = 8  # groups
GS = C // G  # 32 channels per group
P = 128
CB = C // P  # 2 channel blocks
EPS = 1e-5
NCHUNK = 512  # query chunk
NJ = N // NCHUNK  # 8
MB = N // P  # 32 key blocks
NP = MB // 2  # 16 key-block pairs
JJ = NCHUNK // P  # 4 query sub-blocks per chunk
VW = 272  # padded v_aug width (DoubleRow pair stride must be %16)
DCOL = 256  # denominator (ones) column index

SQA = float(np.sqrt(8.0 * np.log2(np.e) / 16.0))  # 0.849329
BS = 20.0  # byte bias: byte = psum + BS
LN2_8 = float(np.log(2.0) / 8.0)
EBIAS = float((BS - 56.0) * np.log(2.0) / 8.0)

# exp engine assignment per pair: True -> ACT, False -> DVE
ACT_PAIRS = tuple(p % 2 == 0 for p in range(NP))

_CACHE: dict = {}


def build_nc(att_reps=1, act_pairs=ACT_PAIRS):
    nc = bacc.Bacc(
        "TRN2",
        target_bir_lowering=False,
        debug=False,
        enable_asserts=False,
        num_devices=B,
    )

    x_d = nc.dram_tensor("x", [C, N], F32, kind="ExternalInput")
    gamma_d = nc.dram_tensor("gamma", [C], F32, kind="ExternalInput")
    beta_d = nc.dram_tensor("beta", [C], F32, kind="ExternalInput")
    w_d = {}
    b_d = {}
    for nm in ("q", "k", "v", "p"):
        w_d[nm] = nc.dram_tensor(f"W{nm}", [C, C], F32, kind="ExternalInput")
        b_d[nm] = nc.dram_tensor(f"b{nm}", [C], F32, kind="ExternalInput")
    out_d = nc.dram_tensor("out", [C, N], F32, kind="ExternalOutput")

    import ml_dtypes

    ident_d = nc.inline_tensor(np.eye(P, dtype=np.float32), name="ident")
    identbf_d = nc.inline_tensor(
        np.eye(P, dtype=ml_dtypes.bfloat16), name="identbf"
    )
    gsum_np = np.zeros((P, G // CB), np.float32)
    for p in range(P):
        gsum_np[p, p // GS] = 1.0 / GS
    gsum_d = nc.inline_tensor(gsum_np, name="gsum")
    gbc_np = np.zeros((G // CB, P), np.float32)
    for p in range(P):
        gbc_np[p // GS, p] = 1.0
    gbc_d = nc.inline_tensor(gbc_np, name="gbc")

    from contextlib import ExitStack

    with tile.TileContext(nc) as tc:
        with ExitStack() as ctx:
            _build_tile(
                ctx, tc, x_d, gamma_d, beta_d, w_d, b_d, out_d, ident_d,
                identbf_d, gsum_d, gbc_d, att_reps, act_pairs,
            )
    nc.compile()
    return nc


def _build_tile(ctx, tc, x_d, gamma_d, beta_d, w_d, b_d, out_d, ident_d,
                identbf_d, gsum_d, gbc_d, att_reps, act_pairs):
    nc = tc.nc

    persist = ctx.enter_context(tc.tile_pool(name="persist", bufs=1))
    staging = ctx.enter_context(tc.tile_pool(name="staging", bufs=4))
    sexp = ctx.enter_context(tc.tile_pool(name="sexp", bufs=36))
    sout = ctx.enter_context(tc.tile_pool(name="sout", bufs=2))
    sanm = ctx.enter_context(tc.tile_pool(name="sanm", bufs=2))
    stmp = ctx.enter_context(tc.tile_pool(name="stmp", bufs=6))
    ps_sc = ctx.enter_context(tc.tile_pool(name="ps_sc", bufs=3, space="PSUM"))
    ps_pv = ctx.enter_context(tc.tile_pool(name="ps_pv", bufs=2, space="PSUM"))

    t_cm = persist.tile([P, CB, N], BF16, tag="t_cm")
    t_r = persist.tile([P, CB, N], BF16, tag="t_r")
    NSUB = N // 512

    # ---- x first (critical path): casting SWDGE DMAs f32 -> bf16 (halves
    # the modeled DMA-bus time; HWDGE stays free so the tiny constants land
    # on the bus first). 8 pieces so bn_stats starts on the first KB.
    XW = 2048
    for q in range(N // XW):
        sl = slice(q * XW, (q + 1) * XW)
        for cb in range(CB):
            nc.gpsimd.dma_start(
                out=t_cm[:, cb, sl], in_=x_d[cb * P : (cb + 1) * P, sl]
            )

    # ---- weights: casting DMAs to bf16 staging ----
    w_dma = {}
    for nm in ("q", "k", "p", "v"):
        w_sb = staging.tile([P, CB, C], BF16, tag="w_stage", name=f"w_sb_{nm}")
        nc.gpsimd.dma_start(
            out=w_sb, in_=w_d[nm][:, :].rearrange("(b p) i -> p b i", p=P)
        )
        w_dma[nm] = w_sb

    # ---- tiny constants on HWDGE (transfer ahead of the big loads) ----
    gsum = persist.tile([P, G // CB], F32, tag="gsum")
    nc.sync.dma_start(out=gsum, in_=gsum_d[:, :])
    gbc = persist.tile([G // CB, P], F32, tag="gbc")
    nc.sync.dma_start(out=gbc, in_=gbc_d[:, :])

    def col_tile(dram_vec, tag, q=nc.sync):
        t = persist.tile([P, CB], F32, tag=tag)
        q.dma_start(out=t, in_=dram_vec[:].rearrange("(b p) -> p b", p=P))
        return t

    gamma_col = col_tile(gamma_d, "gamma_col")
    beta_col = col_tile(beta_d, "beta_col")
    # NOTE: bk is dropped entirely -- adding bk to K shifts each query's
    # scores by a per-query constant along the key axis, which softmax
    # cancels exactly.
    bq_row = persist.tile([1, C], F32, tag="bq_row")
    nc.scalar.dma_start(
        out=bq_row, in_=b_d["q"][:].rearrange("(a c) -> a c", a=1)
    )
    ident = persist.tile([P, P], F32, tag="ident")
    nc.sync.dma_start(out=ident, in_=ident_d[:, :])
    ident_bf = persist.tile([P, P], BF16, tag="ident_bf")
    nc.scalar.dma_start(out=ident_bf, in_=identbf_d[:, :])
    ident_r = persist.tile([P, P], F32R, tag="ident_r")
    nc.scalar.copy(out=ident_r, in_=ident)
    bv_col = col_tile(b_d["v"], "bv_col", q=nc.scalar)
    bp_col = col_tile(b_d["p"], "bp_col", q=nc.scalar)

    # q-bias as a bf16 row (folded into the q PSUM via a rank-1 matmul)
    bq_bf = persist.tile([1, C], BF16, tag="bq_bf")
    nc.gpsimd.tensor_copy(out=bq_bf, in_=bq_row)
    ones512_bf = persist.tile([1, NCHUNK], BF16, tag="ones512_bf")
    nc.gpsimd.memset(ones512_bf, 1.0)

    ebias_col = persist.tile([P, 1], F32, tag="ebias_col")
    nc.gpsimd.memset(ebias_col, EBIAS)

    # ---- GN stats first in the DVE queue (don't block behind consts) ----
    all_stats = []
    for cb in range(CB):
        stats = stmp.tile([P, NSUB, 6], F32, tag="gn_stats", name=f"stats_{cb}")
        all_stats.append(stats)
    gn_cols = []

    def gn_chain(cb):
        """cross-partition group reduction -> per-channel scale/shift."""
        stats = all_stats[cb]
        mv = stmp.tile([P, 2], F32, tag="gn_mv")
        nc.vector.bn_aggr(out=mv, in_=stats)
        stats2 = stmp.tile([P, 2], F32, tag="gn_stats2")
        nc.vector.tensor_copy(out=stats2[:, 0:1], in_=mv[:, 0:1])
        nc.vector.tensor_tensor(
            out=stats2[:, 1:2], in0=mv[:, 0:1], in1=mv[:, 0:1], op=OP.mult
        )
        nc.vector.tensor_add(out=stats2[:, 1:2], in0=stats2[:, 1:2], in1=mv[:, 1:2])
        gps = ps_pv.tile([G // CB, 2], F32, tag="ps_pv", name=f"gps_{cb}")
        nc.tensor.matmul(gps, lhsT=gsum, rhs=stats2, start=True, stop=True)
        gsb = stmp.tile([G // CB, 2], F32, tag="gn_gsb")
        nc.vector.tensor_copy(out=gsb, in_=gps)
        gpack = stmp.tile([G // CB, 2], F32, tag="gn_gpack")
        nc.vector.tensor_copy(out=gpack[:, 0:1], in_=gsb[:, 0:1])
        gvar = stmp.tile([G // CB, 1], F32, tag="gn_gvar")
        nc.vector.tensor_tensor(out=gvar, in0=gsb[:, 0:1], in1=gsb[:, 0:1], op=OP.mult)
        nc.vector.tensor_tensor(out=gvar, in0=gsb[:, 1:2], in1=gvar, op=OP.subtract)
        nc.vector.tensor_scalar_add(out=gvar, in0=gvar, scalar1=EPS)
        # rstd = rsqrt(gvar) fully on DVE (bit-trick seed + 3 Newton steps)
        # to keep the ACT Sqrt table load off the critical path.
        y = stmp.tile([G // CB, 1], F32, tag="gn_y")
        nc.vector.tensor_scalar(
            out=y.bitcast(mybir.dt.int32), in0=gvar.bitcast(mybir.dt.int32),
            scalar1=1, scalar2=None, op0=OP.logical_shift_right, op1=OP.bypass,
        )
        nc.vector.tensor_scalar(
            out=y.bitcast(mybir.dt.int32), in0=y.bitcast(mybir.dt.int32),
            scalar1=-1, scalar2=0x5F3759DF, op0=OP.mult, op1=OP.add,
        )
        vh = stmp.tile([G // CB, 1], F32, tag="gn_vh")
        nc.vector.tensor_scalar_mul(out=vh, in0=gvar, scalar1=-0.5)
        for _ in range(3):
            yy = stmp.tile([G // CB, 1], F32, tag="gn_yy")
            nc.vector.tensor_tensor(out=yy, in0=y, in1=y, op=OP.mult)
            nc.vector.tensor_tensor(out=yy, in0=yy, in1=vh, op=OP.mult)
            nc.vector.tensor_scalar_add(out=yy, in0=yy, scalar1=1.5)
            nc.vector.tensor_tensor(out=y, in0=y, in1=yy, op=OP.mult)
        nc.vector.tensor_copy(out=gpack[:, 1:2], in_=y)
        bps = ps_pv.tile([P, 2], F32, tag="ps_pv", name=f"gbps_{cb}")
        nc.tensor.matmul(bps, lhsT=gbc, rhs=gpack, start=True, stop=True)
        s_col = stmp.tile([P, 1], F32, tag="gn_scol")
        nc.vector.tensor_tensor(
            out=s_col, in0=bps[:, 1:2], in1=gamma_col[:, cb : cb + 1], op=OP.mult
        )
        b_col = stmp.tile([P, 1], F32, tag="gn_bcol")
        nc.vector.tensor_tensor(out=b_col, in0=bps[:, 0:1], in1=s_col, op=OP.mult)
        nc.vector.tensor_tensor(
            out=b_col, in0=beta_col[:, cb : cb + 1], in1=b_col, op=OP.subtract
        )
        gn_cols.append([s_col, b_col, None])

    # piece order matches the x DMA arrival order; cb0's reduction chain
    # runs while cb1's last stats batch is still on the DVE queue
    for q in range(N // XW):
        for cb in range(CB):
            if q == N // XW - 1 and cb == 1:
                gn_chain(0)
            for s in range(4 * q, 4 * q + 4):
                nc.vector.bn_stats(
                    out=all_stats[cb][:, s, :],
                    in_=t_cm[:, cb, s * 512 : (s + 1) * 512],
                )
    gn_chain(1)

    # ---- weights: bf16 transpose via regular matmul against bf16 ident:
    # out = w_slice.T @ I. Copies PSUM -> bf16 SBUF on ACT (idle early).
    wT = {}
    w_stage = w_dma
    for nm in ("q", "k", "p"):
        w_sb = w_stage[nm]
        wt = persist.tile([P, CB, C], BF16, tag=f"w{nm}T")
        for b1 in range(CB):  # c_out block
            for b2 in range(CB):  # c_in block
                tp = ps_sc.tile([P, P], F32, tag="ps_sc", name=f"wtp_{nm}_{b1}_{b2}")
                nc.tensor.matmul(
                    tp, lhsT=w_sb[:, b1, b2 * P : (b2 + 1) * P], rhs=ident_bf,
                    start=True, stop=True,
                )
                nc.scalar.copy(out=wt[:, b2, b1 * P : (b1 + 1) * P], in_=tp)
        wT[nm] = wt

    # ---- WvpT = (Wp @ Wv).T in bf16: lhsT = Wv (natural) @ rhs WpT ----
    wv_r = w_stage["v"]
    wvpT = persist.tile([P, CB, C], BF16, tag="wvpT")
    for ci_b in range(CB):
        pvp = ps_sc.tile([P, C], F32, tag="ps_sc", name=f"pvp_{ci_b}")
        for cm_b in range(CB):
            nc.tensor.matmul(
                pvp,
                lhsT=wv_r[:, cm_b, ci_b * P : (ci_b + 1) * P],
                rhs=wT["p"][:, cm_b, :],
                start=(cm_b == 0),
                stop=(cm_b == CB - 1),
            )
        nc.scalar.copy(out=wvpT[:, ci_b, :], in_=pvp)

    # ---- bv2_col = (Wp @ bv) as per-channel column [P, CB] ----
    bv_bf = persist.tile([P, CB], BF16, tag="bv_bf")
    nc.scalar.copy(out=bv_bf, in_=bv_col)
    bv2_col = persist.tile([P, CB], F32, tag="bv2_col")
    for cb in range(CB):
        bps = ps_pv.tile([P, 1], F32, tag="ps_pv", name=f"bv2ps_{cb}")
        for cm_b in range(CB):
            nc.tensor.matmul(
                bps,
                lhsT=wT["p"][:, cm_b, cb * P : (cb + 1) * P],
                rhs=bv_bf[:, cm_b : cm_b + 1],
                start=(cm_b == 0),
                stop=(cm_b == CB - 1),
            )
        nc.vector.tensor_copy(out=bv2_col[:, cb : cb + 1], in_=bps)

    # ---- residual bias columns (need bv2_col, so emitted after it) ----
    for cb in range(CB):
        s_col, b_col, _ = gn_cols[cb]
        bpb_col = stmp.tile([P, 1], F32, tag="gn_bpb")
        nc.vector.tensor_add(out=bpb_col, in0=b_col, in1=bp_col[:, cb : cb + 1])
        nc.vector.tensor_add(out=bpb_col, in0=bpb_col, in1=bv2_col[:, cb : cb + 1])
        gn_cols[cb][2] = bpb_col

    # ---- t_r (bf16 normalized tokens): DVE cb0 / ACT cb1 early, Pool late
    # t_r entirely on DVE: bf16-in/bf16-out tensor_scalar hits the 4x DVE
    # mode (~0.13 ns/elem), far cheaper than ACT or Pool.
    s0, b0, _ = gn_cols[0]
    s1, b1, _ = gn_cols[1]
    for u in range(2):
        asl = slice(u * 2048, (u + 1) * 2048)
        for cb, (sc_, bc_) in enumerate(((s0, b0), (s1, b1))):
            nc.vector.tensor_scalar(
                out=t_r[:, cb, asl], in0=t_cm[:, cb, asl], scalar1=sc_,
                scalar2=bc_, op0=OP.mult, op1=OP.add,
            )

    # fp32 residual apply on Pool: t_cm := x*s + (b + bp + bv2)
    def apply_chunk(ch):
        asl = slice(ch * 512, (ch + 1) * 512)
        for cb in range(CB):
            s_col, _, bpb_col = gn_cols[cb]
            nc.gpsimd.tensor_scalar(
                out=t_cm[:, cb, asl], in0=t_cm[:, cb, asl], scalar1=s_col,
                scalar2=bpb_col, op0=OP.mult, op1=OP.add,
            )

    # ---- Q, K (fp8, pre-scaled), V2 (fp8 + ones col) ----
    q_cm = persist.tile([P, CB, N], FP8, tag="q_cm")
    k_cm = persist.tile([P, CB, N], FP8, tag="k_cm")
    v_aug = persist.tile([P, MB, VW], FP8, tag="v_aug")
    nc.gpsimd.memset(v_aug[:, :, DCOL:VW], 0.0)
    nc.gpsimd.memset(v_aug[:, :, DCOL : DCOL + 1], 1.0)

    def qk_chunk(nm, ch, dve):
        """project tokens [ch*512:(ch+1)*512] -> q_cm/k_cm fp8.

        k drops its bias (softmax-invariant); q folds bq into the PSUM via
        a rank-1 matmul. The epilogue is then a single uniform-scale
        convert over both channel blocks."""
        sl = slice(ch * NCHUNK, (ch + 1) * NCHUNK)
        wt = wT[nm]
        dst = q_cm if nm == "q" else k_cm
        pq = ps_sc.tile([P, CB, NCHUNK], F32, tag="ps_sc", name=f"p{nm}_{ch}")
        for cb in range(CB):
            last = CB - 1 if nm == "k" else CB
            for ci in range(CB):
                nc.tensor.matmul(
                    pq[:, cb, :],
                    lhsT=wt[:, ci, cb * P : (cb + 1) * P],
                    rhs=t_r[:, ci, sl],
                    start=(ci == 0),
                    stop=(ci == last),
                )
            if nm == "q":
                nc.tensor.matmul(
                    pq[:, cb, :],
                    lhsT=bq_bf[:, cb * P : (cb + 1) * P],
                    rhs=ones512_bf,
                    start=False,
                    stop=True,
                )
        if dve:
            nc.vector.tensor_scalar_mul(out=dst[:, :, sl], in0=pq, scalar1=SQA)
        else:
            nc.scalar.mul(out=dst[:, :, sl], in_=pq, mul=SQA)

    def v_group(g, dve):
        """v2 for key blocks 4g..4g+3 (pairs 2g, 2g+1) -> v_aug."""
        pv = ps_sc.tile([P, 4, C], F32, tag="ps_sc", name=f"pv_{g}")
        for h in range(4):
            nb = 4 * g + h
            for ci in range(CB):
                nc.tensor.matmul(
                    pv[:, h, :],
                    lhsT=t_r[:, ci, nb * P : (nb + 1) * P],
                    rhs=wvpT[:, ci, :],
                    start=(ci == 0),
                    stop=(ci == CB - 1),
                )
        dst = v_aug[:, 4 * g : 4 * g + 4, 0:C]
        if dve:
            nc.vector.tensor_copy(out=dst, in_=pv)
        else:
            nc.scalar.copy(out=dst, in_=pv)

    # ---- attention ----
    def scores_pair(j, pp):
        jsl = slice((j % NJ) * NCHUNK, (j % NJ + 1) * NCHUNK)
        sc = ps_sc.tile([P, 2, NCHUNK], F32, tag="ps_sc", name=f"sc_{j}_{pp}")
        for h in range(2):
            i = 2 * pp + h
            nc.tensor.matmul(
                sc[:, h, :],
                lhsT=k_cm[:, :, i * P : (i + 1) * P],
                rhs=q_cm[:, :, jsl],
                start=True,
                stop=True,
                perf_mode=DR,
            )
        ex = sexp.tile([P, 2, NCHUNK], FP8, tag="exp")
        if act_pairs[pp] or (j > 0 and pp == NP - 1):
            nc.scalar.activation(
                out=ex, in_=sc, func=AF.Exp, scale=LN2_8, bias=ebias_col
            )
        else:
            nc.vector.tensor_scalar(
                out=ex.bitcast(U8), in0=sc, scalar1=BS, scalar2=0.0,
                op0=OP.add, op1=OP.max,
            )
        return ex

    def pv_half(pvAB, jlo, pp, ex):
        """PV for query sub-blocks jlo, jlo+1 (one pass) over pair pp."""
        for u in range(2):
            jj = jlo + u
            nc.tensor.matmul(
                pvAB[u],
                lhsT=ex[:, :, jj * P : (jj + 1) * P],
                rhs=v_aug[:, 2 * pp : 2 * pp + 2, :],
                start=(pp == 0),
                stop=(pp == NP - 1),
                perf_mode=DR,
            )

    def pv_half_tiles(j, jlo):
        return [
            ps_pv.tile([P, VW], F32, tag="ps_pv", name=f"pv_ps_{j}_{jlo + u}")
            for u in range(2)
        ]

    def norm_half(j, anm, pvAB, jlo, dve=False):
        for u in range(2):
            jj = jlo + u
            rec = stmp.tile([P, 1], F32, tag="rec")
            nc.vector.reciprocal(out=rec, in_=pvAB[u][:, DCOL : DCOL + 1])
            if dve:
                nc.vector.tensor_scalar_mul(
                    out=anm[:, jj, :], in0=pvAB[u][:, 0:C], scalar1=rec
                )
            else:
                nc.scalar.activation(
                    out=anm[:, jj, :], in_=pvAB[u][:, 0:C], func=AF.Copy,
                    scale=rec, bias=0.0,
                )

    def tail_co(j, anm, co):
        """transpose + residual-add + (after co1) store for chunk j."""
        jn = j % NJ
        jsl = slice(jn * NCHUNK, (jn + 1) * NCHUNK)
        if co == 0:
            obs = sout.tile([P, CB, NCHUNK], F32, tag="out", name=f"ob_{j}")
            tail_obs[j] = obs
        else:
            obs = tail_obs[j]
        tp = ps_pv.tile([P, NCHUNK], F32R, tag="ps_pv", name=f"tp_{j}_{co}")
        for jj in range(JJ):
            nc.tensor.transpose(
                tp[:, jj * P : (jj + 1) * P],
                anm[:, jj, co * P : (co + 1) * P],
                ident_r,
            )
        nc.vector.tensor_add(out=obs[:, co, :], in0=tp, in1=t_cm[:, co, jsl])
        if co == CB - 1:
            nc.sync.dma_start(
                out=out_d[:, jsl].rearrange("(b p) n -> p b n", p=P), in_=obs
            )

    tail_obs = {}

    def chunk_tail(prev):
        """pass-2 PV + epilogue for the previous chunk, as one block (used
        for chunk 0's predecessor slot and the final flush)."""
        if prev is None:
            return
        j, exs, anm = prev
        pvCD = pv_half_tiles(j, 2)
        for pp in range(NP):
            pv_half(pvCD, 2, pp, exs[pp])
        norm_half(j, anm, pvCD, 2)
        tail_co(j, anm, 0)
        tail_co(j, anm, 1)

    prev = None  # (j, exs, anm) awaiting pass-2 + epilogue

    # ---- chunk 0: prime K/V for ch 0-1, then one K-or-V production per
    # scores slot (SBUF-resident k_cm/v_aug decouple the streams).
    qk_chunk("q", 0, dve=True)
    qk_chunk("k", 0, dve=False)
    v_group(0, dve=True)
    qk_chunk("k", 1, dve=True)
    qk_chunk("k", 2, dve=False)
    v_group(1, dve=False)
    exs0 = {}
    pvAB = None
    kv_sched = [("v", 2), ("k", 3), ("v", 3), ("k", 4), ("v", 4), ("k", 5),
                ("v", 5), ("k", 6), ("v", 6), ("k", 7), ("v", 7)]
    for pp in range(NP):
        exs0[pp] = scores_pair(0, pp)
        if pp < len(kv_sched):
            kind, ch = kv_sched[pp]
            if kind == "k":
                qk_chunk("k", ch, dve=(ch % 2 == 1))
            else:
                v_group(ch, dve=(ch % 2 == 0))
        elif pp == 12:
            qk_chunk("q", 1, dve=False)
        if pp == 3:
            pvAB = pv_half_tiles(0, 0)
        if pp >= 3:
            pv_half(pvAB, 0, pp - 3, exs0[pp - 3])
    for pq_ in range(NP - 3, NP):
        pv_half(pvAB, 0, pq_, exs0[pq_])
    apply_chunk(0)
    anm0 = sanm.tile([P, JJ, C], F32R, tag="anm", name="anm_0")
    norm_half(0, anm0, pvAB, 0)
    prev = (0, exs0, anm0)

    # ---- remaining chunks: the scores/exp stream for chunk j carries the
    # deferred pass-2 + epilogue of chunk j-1, interleaved finely so the
    # exp engines never wait behind bulk PE work in issue order.
    for j in range(1, NJ * att_reps):
        if j < NJ:
            apply_chunk(j)
        pj, pexs, panm = prev
        pvCD = pv_half_tiles(pj, 2)
        exs = {}
        pvAB = None
        for pp in range(NP):
            exs[pp] = scores_pair(j, pp)
            if pp < 8:
                pv_half(pvCD, 2, 2 * pp, pexs[2 * pp])
                pv_half(pvCD, 2, 2 * pp + 1, pexs[2 * pp + 1])
            elif pp == 10:
                if j + 1 < NJ:
                    qk_chunk("q", j + 1, dve=False)
            elif pp == 11:
                norm_half(pj, panm, pvCD, 2, dve=True)
                tail_co(pj, panm, 0)
            elif pp == 12:
                tail_co(pj, panm, 1)
            elif pp == 13:
                pvAB = pv_half_tiles(j, 0)
                pv_half(pvAB, 0, 0, exs[0])
                pv_half(pvAB, 0, 1, exs[1])
            elif pp >= 14:  # pairs 2..5 over slots 14,15
                lo = 2 * (pp - 13)
                pv_half(pvAB, 0, lo, exs[lo])
                pv_half(pvAB, 0, lo + 1, exs[lo + 1])
        for pq_ in range(6, NP):
            pv_half(pvAB, 0, pq_, exs[pq_])
        anm = sanm.tile([P, JJ, C], F32R, tag="anm", name=f"anm_{j}")
        norm_half(j, anm, pvAB, 0)
        prev = (j, exs, anm)

    chunk_tail(prev)


def kernel(x, gamma, beta, Wq, bq, Wk, bk, Wv, bv, Wp, bp):
    if "nc" not in _CACHE:
        _CACHE["nc"] = build_nc()
    nc = _CACHE["nc"]

    x = np.ascontiguousarray(np.asarray(x, dtype=np.float32)).reshape(B, C, N)
    common = {
        "gamma": np.asarray(gamma, np.float32),
        "beta": np.asarray(beta, np.float32),
        "Wq": np.asarray(Wq, np.float32),
        "bq": np.asarray(bq, np.float32),
        "Wk": np.asarray(Wk, np.float32),
        "bk": np.asarray(bk, np.float32),
        "Wv": np.asarray(Wv, np.float32),
        "bv": np.asarray(bv, np.float32),
        "Wp": np.asarray(Wp, np.float32),
        "bp": np.asarray(bp, np.float32),
    }
    in_maps = [{"x": x[b], **common} for b in range(B)]
    res = bass_utils.run_bass_kernel_spmd(nc, in_maps, core_ids=list(range(B)))
    out = np.stack([res.results[b]["out"] for b in range(B)])
    return out.reshape(B, C, H, W)
